# revision 1
# baseline (speedup 1.0000x reference)
"""GAT 2-layer encoder on 8 Trainium2 NeuronCores.

Reference computation: layer 1 = GAT conv over edge_index[:, :500] (weights W1),
layer 2 = GAT conv over edge_index[:, 500:] (weights W2).

Strategy:
  - Layer-1 output x1 differs from b1 only on the <=500 distinct dsts of the
    first 500 edges ("specials").  By linearity, layer 2's weighted aggregation
    commutes with the W2 transform, so layer 2 gathers x1-space rows and the
    gather table collapses to <=501 distinct 512B rows [x1 | asrc2 | adst2 | pad]
    (row 0 = default b1 row, rows 1..K = specials).  Indices then fit in int16
    for dma_gather.
  - Sharding: dst-range partition of the 1.6M layer-2 edges across 8 cores (no
    collectives; layer 1 + table build replicated on every core, it is tiny).
  - Per core: dsts sorted by in-degree, grouped into blocks of 128 (one dst per
    SBUF partition, its edges along the free dim, padded to the block max degree
    L).  One dma_gather per superblock fetches one 512B row per edge slot.
    Segment softmax = per-partition free-dim ops (DVE/ACT), weighted sum = DVE
    mul + strided reduce, final out = PE matmul [msgT;1] @ [W2;b2].
"""

import sys

sys.path.insert(0, "/opt/trn_rl_repo")

from contextlib import ExitStack

import numpy as np

import concourse.bacc as bacc
import concourse.bass as bass
import concourse.mybir as mybir
import concourse.tile as tile
from concourse.bass_utils import run_bass_kernel_spmd
from concourse.masks import make_identity

F32 = mybir.dt.float32
I16 = mybir.dt.int16
I32 = mybir.dt.int32
AF = mybir.ActivationFunctionType
OP = mybir.AluOpType

N = 100000
D = 64
NCORES = 8
NPC = N // NCORES          # dst nodes per core
P = 128
NSPLIT = 500               # first 500 edges -> layer 1
SMAX = 80                  # max edge-slots per superblock (SBUF budget)
NEG_SLOPE = 0.2
EPS = 1e-16
BIG = 200.0                # score shift so padded slots underflow exp to 0.0
GCHUNK = 32                # slots per packed gather call
PW = 4                     # slots packed per gather descriptor (PW*512B rows)


def _wrap16(flat):
    """int16 stream [n] (n%16==0) -> dma_gather idx tile [128, n//16]."""
    w = flat.reshape(-1, 16).T
    return np.ascontiguousarray(np.tile(w, (8, 1)).astype(np.int16))


def _grid(deg_sorted_max, npos):
    """Block structure from the (cross-core max) descending degree profile.

    Returns (L_b list, superblocks, groups):
      superblocks: dicts {b0, b1, S (slots), slot0}
      groups: dicts {sb, b0, B, L, slot_off (slots from sb start)}
    """
    nblocks = npos // P
    L = [max(int(deg_sorted_max[b * P]), 1) for b in range(nblocks)]
    sbs = []
    b = 0
    while b < nblocks:
        s = 0
        b0 = b
        while b < nblocks and (b - b0) < 16 and s + L[b] <= max(SMAX, L[b0]):
            s += L[b]
            b += 1
        sbs.append({"b0": b0, "b1": b, "S": s})
    slot0 = 0
    for sb in sbs:
        sb["slot0"] = slot0
        slot0 += sb["S"]
    groups = []
    for si, sb in enumerate(sbs):
        off = 0
        b = sb["b0"]
        while b < sb["b1"]:
            b0 = b
            while b < sb["b1"] and L[b] == L[b0]:
                b += 1
            groups.append({"sb": si, "b0": b0, "B": b - b0, "L": L[b0], "slot_off": off})
            off += (b - b0) * L[b0]
    return L, sbs, groups


VTAB = 1024               # gather table rows (specials + default replicas)


def _edge_streams(src, dst_local, rowmap_vals, npos, npc, Lb, sbs, repl_lo):
    """Per-partition edge grid for one core.

    Returns (eidx [128, 8*S_total] i16, mask [128, S_total] f32,
             degpos [128, nblocks] f32, order [npc])."""
    nblocks = npos // P
    deg = np.bincount(dst_local, minlength=npc)
    order = np.argsort(-deg, kind="stable")
    deg_sorted = deg[order]
    rank = np.empty(npc, np.int64)
    rank[order] = np.arange(npc)
    pos = rank[dst_local]
    pe = np.argsort(pos, kind="stable")
    pos_s = pos[pe]
    val_s = rowmap_vals[pe]
    start_of_pos = np.searchsorted(pos_s, np.arange(npos))
    k = np.arange(len(pos_s)) - start_of_pos[pos_s]
    blk = pos_s // P
    prt = pos_s % P
    slot_base = np.concatenate([[0], np.cumsum(Lb)])[:-1]
    s_global = slot_base[blk] + k
    S_total = int(sum(Lb))
    flat_j = s_global * P + prt
    rng = np.random.default_rng(12345)
    idxflat = rng.integers(repl_lo, VTAB, S_total * P).astype(np.int16)
    vs = val_s.astype(np.int16)
    zz = vs == 0
    vs[zz] = rng.integers(repl_lo, VTAB, int(zz.sum())).astype(np.int16)
    idxflat[flat_j] = vs
    maskflat = np.zeros(S_total * P, np.float32)
    maskflat[flat_j] = 1.0
    mask = np.ascontiguousarray(maskflat.reshape(S_total, P).T)
    eidx = np.concatenate(
        [_wrap16(idxflat[sb["slot0"] * P:(sb["slot0"] + sb["S"]) * P]) for sb in sbs],
        axis=1,
    )
    degpad = np.zeros(npos, np.float32)
    degpad[:npc] = deg_sorted
    degpos = np.ascontiguousarray((degpad > 0).astype(np.float32).reshape(nblocks, P).T)
    return eidx, mask, degpos, order, idxflat


def prep(inputs):
    """Host-side index prep (pure index computation, no feature values)."""
    ei = np.asarray(inputs["edge_index"])
    src = ei[0].astype(np.int64)
    dst = ei[1].astype(np.int64)
    s1, d1 = src[:NSPLIT], dst[:NSPLIT]
    s2, d2 = src[NSPLIT:], dst[NSPLIT:]

    # ---- layer 1 structure ----
    specials, deg1 = np.unique(d1, return_counts=True)
    K = len(specials)
    order1 = np.argsort(-deg1, kind="stable")
    spec_by_pos = specials[order1]          # grid position q -> node, table row q+1
    rowmap = np.zeros(N, np.int16)
    rowmap[spec_by_pos] = np.arange(1, K + 1)
    K1pos = K + 1                            # one guaranteed pad slot (default row)
    nblk1 = (K1pos + P - 1) // P
    npos1 = nblk1 * P

    U = np.unique(np.concatenate([s1, d1]))
    nU = len(U)
    nUt = (nU + P - 1) // P
    uidx = np.zeros((P, nUt), np.int32)
    upad = np.zeros(nUt * P, np.int64)
    upad[:nU] = U
    uidx[:, :] = upad.reshape(nUt, P).T
    uindex = np.zeros(N, np.int64)
    uindex[U] = np.arange(nU)

    # layer-1 edge grid (dst -> grid position via rank over specials)
    rank1 = np.empty(K, np.int64)
    rank1[order1] = np.arange(K)
    d1pos = rank1[np.searchsorted(specials, d1)]
    deg1_sorted = np.zeros(npos1, np.int64)
    deg1_sorted[:K] = deg1[order1]
    L1, sbs1, groups1 = _grid(deg1_sorted, npos1)
    S1 = int(sum(L1))
    # edge stream for layer 1 (single "core")
    pe = np.argsort(d1pos, kind="stable")
    pos_s = d1pos[pe]
    val_s = uindex[s1[pe]].astype(np.int16)
    start_of_pos = np.searchsorted(pos_s, np.arange(npos1))
    k = np.arange(len(pos_s)) - start_of_pos[pos_s]
    slot_base = np.concatenate([[0], np.cumsum(L1)])[:-1]
    flat_j = (slot_base[pos_s // P] + k) * P + (pos_s % P)
    idxflat = np.zeros(S1 * P, np.int16)
    idxflat[flat_j] = val_s
    maskflat = np.zeros(S1 * P, np.float32)
    maskflat[flat_j] = 1.0
    l1_mask = np.ascontiguousarray(maskflat.reshape(S1, P).T)
    l1_eidx = np.concatenate(
        [_wrap16(idxflat[sb["slot0"] * P:(sb["slot0"] + sb["S"]) * P]) for sb in sbs1],
        axis=1,
    )
    dv1 = np.zeros(npos1, np.int16)
    dv1[:K] = uindex[spec_by_pos]
    l1_didx = np.concatenate(
        [_wrap16(dv1[P * sb["b0"]:P * sb["b1"]]) for sb in sbs1], axis=1
    )
    dp1 = np.zeros(npos1, np.float32)
    dp1[:K] = (deg1[order1] > 0)
    l1_degpos = np.ascontiguousarray(dp1.reshape(nblk1, P).T)

    # ---- layer 2 structure ----
    npos = ((NPC + P - 1) // P) * P
    core_dat = []
    deg_sorted_all = np.zeros(npos, np.int64)
    for c in range(NCORES):
        sel = (d2 >= c * NPC) & (d2 < (c + 1) * NPC)
        dl = d2[sel] - c * NPC
        sl = s2[sel]
        deg = np.bincount(dl, minlength=NPC)
        ds = np.sort(deg)[::-1]
        m = min(NPC, npos)
        deg_sorted_all[:m] = np.maximum(deg_sorted_all[:m], ds[:m])
        core_dat.append((sl, dl))
    L2, sbs2, groups2 = _grid(deg_sorted_all, npos)
    dcol = 0
    for sb in sbs2:
        nblk_sb = sb["b1"] - sb["b0"]
        sb["nb4"] = ((nblk_sb + PW - 1) // PW) * PW
        sb["dcol0"] = dcol
        dcol += sb["nb4"] // PW
    dtot = dcol
    # force slot-count per superblock to a multiple of PW so rows pack cleanly
    for sb in sbs2:
        r = (-sb["S"]) % PW
        if r:
            L2[sb["b1"] - 1] += r
            sb["S"] += r
    slot0 = 0
    for sb in sbs2:
        sb["slot0"] = slot0
        slot0 += sb["S"]
    groups2 = []
    for si, sb in enumerate(sbs2):
        off = 0
        b = sb["b0"]
        while b < sb["b1"]:
            b0 = b
            while b < sb["b1"] and L2[b] == L2[b0]:
                b += 1
            groups2.append({"sb": si, "b0": b0, "B": b - b0, "L": L2[b0],
                            "slot_off": off})
            off += (b - b0) * L2[b0]
    S2 = int(sum(L2))
    nblk2 = npos // P

    cores = []
    lo_pack = (K + PW) // PW    # first all-default packed row in the packed view
    for c in range(NCORES):
        sl, dl = core_dat[c]
        eidx, mask, degpos, order, idxflat = _edge_streams(
            sl, dl, rowmap[sl], npos, NPC, L2, sbs2, K + 1
        )
        # pack PW consecutive slots per partition; all-default packs read one
        # PW*512B replica row, mixed packs read an on-device-built pairfix row
        rngp = np.random.default_rng(4242 + c)
        pidx_segs = []
        pfix_vals = []
        for sb in sbs2:
            s0, S = sb["slot0"], sb["S"]
            iv = idxflat[s0 * P:(s0 + S) * P].reshape(S // PW, PW, P)
            pp = rngp.integers(lo_pack, VTAB // PW,
                               (S // PW, P)).astype(np.int16)
            mixed = (iv <= K).any(axis=1)
            nm = int(mixed.sum())
            if nm:
                pp[mixed] = (VTAB // PW + len(pfix_vals) // PW
                             + np.arange(nm)).astype(np.int16)
                mv = np.moveaxis(iv, 1, 2)[mixed].reshape(-1)
                pfix_vals.extend(mv.tolist())
            pidx_segs.append(_wrap16(pp.reshape(-1)))
        pidx = np.concatenate(pidx_segs, axis=1)
        rngd = np.random.default_rng(777 + c)
        dv = rngd.integers(K + 1, VTAB, npos).astype(np.int16)
        dvr = rowmap[c * NPC + order]
        dz = dvr == 0
        dvr = dvr.copy()
        dvr[dz] = rngd.integers(K + 1, VTAB, int(dz.sum())).astype(np.int16)
        dv[:NPC] = dvr
        dsegs = []
        for sb in sbs2:
            nblk_sb = sb["b1"] - sb["b0"]
            nb4 = sb["nb4"]
            vals = np.full((nb4, P), 0, np.int16)
            vals[:nblk_sb] = dv[P * sb["b0"]:P * sb["b1"]].reshape(nblk_sb, P)
            if nb4 > nblk_sb:
                vals[nblk_sb:] = rngd.integers(
                    K + 1, VTAB, (nb4 - nblk_sb, P)).astype(np.int16)
            v4 = vals.reshape(nb4 // PW, PW, P)
            pp = rngd.integers(lo_pack, VTAB // PW,
                               (nb4 // PW, P)).astype(np.int16)
            mixed = (v4 <= K).any(axis=1)
            nm = int(mixed.sum())
            if nm:
                pp[mixed] = (VTAB // PW + len(pfix_vals) // PW
                             + np.arange(nm)).astype(np.int16)
                pfix_vals.extend(np.moveaxis(v4, 1, 2)[mixed].reshape(-1).tolist())
            dsegs.append(_wrap16(pp.reshape(-1)))
        didx = np.concatenate(dsegs, axis=1)
        cores.append({"eidx": eidx, "mask": mask, "degpos": degpos,
                      "didx": didx, "order": order, "pidx": pidx,
                      "pfix": np.asarray(pfix_vals, np.int16)})
    # common pairfix region size across cores (SPMD program is shared)
    npf = max((len(c["pfix"]) for c in cores), default=0)
    Spf = max((npf + P - 1) // P, 1)
    for c in cores:
        pf = np.zeros(Spf * P, np.int16)
        pf[:len(c["pfix"])] = c["pfix"]
        c["pfidx"] = _wrap16(pf)

    meta = {
        "K": K, "K1pos": K1pos, "nblk1": nblk1, "nU": nU, "nUt": nUt,
        "L1": L1, "sbs1": sbs1, "groups1": groups1, "S1": S1,
        "L2": L2, "sbs2": sbs2, "groups2": groups2, "S2": S2, "nblk2": nblk2,
        "npos": npos, "Spf": Spf, "dtot": dtot,
    }
    l1 = {"uidx": uidx, "l1_eidx": l1_eidx, "l1_didx": l1_didx,
          "l1_mask": l1_mask, "l1_degpos": l1_degpos}
    return meta, l1, cores


def _emit_group(nc, gw, Gap, mask_ap, adst_ap, degpos_ap, B, L):
    """Segment softmax + weighted sum for B blocks of equal padded degree L.

    Gap: AP view [128, B*L, 128] of the gathered rows (slot-flat).
    Returns msg tile [128, B, 64]."""
    BL = B * L
    asrc = Gap[:, :, 64:65].rearrange("p s o -> p (s o)")        # [128, BL]
    s_t = gw.tile([P, B, L], F32, tag="s_t")
    nc.vector.tensor_tensor(s_t[:], asrc, adst_ap.to_broadcast((P, B, L)),
                            op=OP.add)
    u_t = gw.tile([P, B, L], F32, tag="u_t")
    nc.vector.scalar_tensor_tensor(u_t[:], s_t[:], NEG_SLOPE, s_t[:],
                                   op0=OP.mult, op1=OP.max)
    e2_t = gw.tile([P, B, L], F32, tag="e2_t")
    nc.vector.scalar_tensor_tensor(e2_t[:], u_t[:], BIG, mask_ap,
                                   op0=OP.add, op1=OP.mult)
    mneg = gw.tile([P, B], F32, tag="mneg")
    nc.vector.tensor_reduce(mneg[:], e2_t[:], axis=mybir.AxisListType.X,
                            op=OP.max, negate=True)
    d_t = gw.tile([P, B, L], F32, tag="d_t")
    nc.vector.tensor_tensor(d_t[:], e2_t[:], mneg[:].to_broadcast((P, B, L)),
                            op=OP.add)
    ex_t = gw.tile([P, B, L], F32, tag="ex_t")
    nc.scalar.activation(ex_t[:], d_t[:], AF.Exp)
    ssum = gw.tile([P, B], F32, tag="ssum")
    nc.vector.tensor_reduce(ssum[:], ex_t[:], axis=mybir.AxisListType.X,
                            op=OP.add)
    sp = gw.tile([P, B], F32, tag="sp")
    nc.vector.tensor_scalar_add(sp[:], ssum[:], EPS)
    rs = gw.tile([P, B], F32, tag="rs")
    nc.vector.reciprocal(rs[:], sp[:])
    rsd = gw.tile([P, B], F32, tag="rsd")
    nc.vector.tensor_tensor(rsd[:], rs[:], degpos_ap, op=OP.mult)
    alpha = gw.tile([P, B, L], F32, tag="alpha")
    nc.vector.tensor_tensor(alpha[:], ex_t[:], rsd[:].to_broadcast((P, B, L)),
                            op=OP.mult)
    wr = gw.tile([P, BL, D], F32, tag="wr")
    nc.vector.tensor_tensor(wr[:], Gap[:, :, 0:D],
                            alpha[:].rearrange("p b l -> p (b l)")
                            .to_broadcast((P, BL, D)), op=OP.mult)
    msg = gw.tile([P, B, D], F32, tag="msg")
    nc.vector.tensor_reduce(msg[:], wr[:].rearrange("p (b l) f -> p b f l", b=B),
                            axis=mybir.AxisListType.X, op=OP.add)
    return msg


def build(meta, repeat=1, limit_sb=None, debug_lvl=3, gchunk=GCHUNK):
    """Build the SPMD Bass program (common across cores)."""
    K = meta["K"]
    nblk1, nUt = meta["nblk1"], meta["nUt"]
    S1, sbs1, groups1, L1 = meta["S1"], meta["sbs1"], meta["groups1"], meta["L1"]
    S2, sbs2, groups2, L2 = meta["S2"], meta["sbs2"], meta["groups2"], meta["L2"]
    nblk2 = meta["nblk2"]

    nc = bacc.Bacc("TRN2", target_bir_lowering=False, debug=False,
                   num_devices=NCORES)
    dt = nc.dram_tensor
    x_in = dt("x_in", [N, D], F32, kind="ExternalInput").ap()
    W1_in = dt("W1_in", [D, D], F32, kind="ExternalInput").ap()
    W1T_in = dt("W1T_in", [D, D], F32, kind="ExternalInput").ap()
    W2_in = dt("W2_in", [D, D], F32, kind="ExternalInput").ap()
    W2T_in = dt("W2T_in", [D, D], F32, kind="ExternalInput").ap()
    av1_in = dt("av1_in", [D, 2], F32, kind="ExternalInput").ap()
    av2_in = dt("av2_in", [D, 2], F32, kind="ExternalInput").ap()
    b1row_in = dt("b1row_in", [1, D], F32, kind="ExternalInput").ap()
    b2row_in = dt("b2row_in", [1, D], F32, kind="ExternalInput").ap()
    b1col_in = dt("b1col_in", [D, 1], F32, kind="ExternalInput").ap()
    uidx_in = dt("uidx_in", [P, nUt], I32, kind="ExternalInput").ap()
    l1_eidx_in = dt("l1_eidx_in", [P, 8 * S1], I16, kind="ExternalInput").ap()
    l1_didx_in = dt("l1_didx_in", [P, 8 * nblk1], I16, kind="ExternalInput").ap()
    l1_mask_in = dt("l1_mask_in", [P, S1], F32, kind="ExternalInput").ap()
    l1_degpos_in = dt("l1_degpos_in", [P, nblk1], F32, kind="ExternalInput").ap()
    Spf = meta["Spf"]
    pidx_in = dt("pidx_in", [P, 8 * (S2 // PW)], I16, kind="ExternalInput").ap()
    pfidx_in = dt("pfidx_in", [P, 8 * Spf], I16, kind="ExternalInput").ap()
    didx_in = dt("didx_in", [P, 8 * meta["dtot"]], I16, kind="ExternalInput").ap()
    mask_in = dt("mask_in", [P, S2], F32, kind="ExternalInput").ap()
    degpos_in = dt("degpos_in", [P, nblk2], F32, kind="ExternalInput").ap()
    out_t = dt("out", [meta["npos"], D], F32, kind="ExternalOutput").ap()

    h1tab = dt("h1tab", [nUt * P, P], F32).ap()
    tab = dt("tab", [VTAB + Spf * P, P], F32).ap()

    with tile.TileContext(nc) as tc, ExitStack() as ctx:
        const = ctx.enter_context(tc.tile_pool(name="const", bufs=1))
        psc_ctx = tc.tile_pool(name="psc", bufs=1, space="PSUM")
        psc = psc_ctx.__enter__()

        ident = const.tile([P, P], F32)
        make_identity(nc, ident[:])

        # ---- weights / augmented matrices ----
        W1s = const.tile([D, D], F32)
        nc.sync.dma_start(W1s[:], W1_in[:])
        W1Ts = const.tile([D, D], F32)
        nc.sync.dma_start(W1Ts[:], W1T_in[:])
        W2s = const.tile([D, D], F32)
        nc.sync.dma_start(W2s[:], W2_in[:])
        W2Ts = const.tile([D, D], F32)
        nc.sync.dma_start(W2Ts[:], W2T_in[:])
        av1s = const.tile([D, 2], F32)
        nc.sync.dma_start(av1s[:], av1_in[:])
        av2s = const.tile([D, 2], F32)
        nc.sync.dma_start(av2s[:], av2_in[:])
        b1cols = const.tile([D, 1], F32)
        nc.sync.dma_start(b1cols[:], b1col_in[:])

        wt1_p = psc.tile([D, 2], F32, space="PSUM")
        nc.tensor.matmul(wt1_p[:], W1Ts[:], av1s[:], start=True, stop=True)
        wt2_p = psc.tile([D, 2], F32, space="PSUM")
        nc.tensor.matmul(wt2_p[:], W2Ts[:], av2s[:], start=True, stop=True)
        wt2s = const.tile([D, 2], F32)
        nc.vector.tensor_copy(wt2s[:], wt2_p[:])

        W1aug = const.tile([D, D + 2], F32)
        nc.vector.tensor_copy(W1aug[:, 0:D], W1s[:])
        nc.vector.tensor_copy(W1aug[:, D:D + 2], wt1_p[:])

        # SPEC [65, 66] = [[I | wt2s | wt2d]; [b1 | b1.wt2s | b1.wt2d]]
        SPEC = const.tile([D + 1, D + 2], F32)
        nc.vector.tensor_copy(SPEC[0:D, 0:D], ident[0:D, 0:D])
        nc.vector.tensor_copy(SPEC[0:D, D:D + 2], wt2s[:])
        nc.sync.dma_start(SPEC[D:D + 1, 0:D], b1row_in[:])
        b1w_p = psc.tile([1, 2], F32, space="PSUM")
        nc.tensor.matmul(b1w_p[:], b1cols[:], wt2s[:], start=True, stop=True)
        nc.vector.tensor_copy(SPEC[D:D + 1, D:D + 2], b1w_p[:])

        W2OUT = const.tile([D + 1, D], F32)
        nc.vector.tensor_copy(W2OUT[0:D, :], W2s[:])
        nc.sync.dma_start(W2OUT[D:D + 1, :], b2row_in[:])

        psc_ctx.__exit__(None, None, None)

        # ---- layer 1: build h1 table for the U endpoint nodes ----
        uidx_s = const.tile([P, nUt], I32)
        nc.sync.dma_start(uidx_s[:], uidx_in[:])
        with tc.tile_pool(name="l1u", bufs=2) as l1u, \
             tc.tile_pool(name="l1up", bufs=2, space="PSUM") as l1up:
            for t in range(nUt):
                xU = l1u.tile([P, D], F32, tag="xU")
                nc.gpsimd.indirect_dma_start(
                    out=xU[:], out_offset=None, in_=x_in[:, :],
                    in_offset=bass.IndirectOffsetOnAxis(ap=uidx_s[:, t:t + 1], axis=0))
                xT_p = l1up.tile([D, P], F32, space="PSUM", tag="xT")
                nc.tensor.transpose(xT_p[:], xU[:], ident[:])
                xT_s = l1u.tile([D, P], F32, tag="xTs")
                nc.vector.tensor_copy(xT_s[:], xT_p[:])
                h_p = l1up.tile([P, D + 2], F32, space="PSUM", tag="h_p")
                nc.tensor.matmul(h_p[:], xT_s[:], W1aug[:], start=True, stop=True)
                h_s = l1u.tile([P, P], F32, tag="h_s")
                nc.scalar.copy(h_s[:, 0:D + 2], h_p[:])
                nc.vector.memset(h_s[:, D + 2:P], 0.0)
                nc.sync.dma_start(h1tab[t * P:(t + 1) * P, :], h_s[:])

        # ---- layer 1 conv -> write table rows ----
        l1_eidx_s = const.tile([P, 8 * S1], I16)
        nc.sync.dma_start(l1_eidx_s[:], l1_eidx_in[:])
        l1_didx_s = const.tile([P, 8 * nblk1], I16)
        nc.sync.dma_start(l1_didx_s[:], l1_didx_in[:])
        l1_mask_s = const.tile([P, S1], F32)
        nc.sync.dma_start(l1_mask_s[:], l1_mask_in[:])
        l1_degpos_s = const.tile([P, nblk1], F32)
        nc.sync.dma_start(l1_degpos_s[:], l1_degpos_in[:])

        with tc.tile_pool(name="l1w", bufs=2) as l1w, \
             tc.tile_pool(name="l1p", bufs=2, space="PSUM") as l1p:
            dr1 = l1w.tile([P, nblk1, P], F32, tag="dr1")
            nc.gpsimd.dma_gather(dr1[:], h1tab[:, :], l1_didx_s[:],
                                 nblk1 * P, nblk1 * P, P, single_packet=False)
            adst1 = l1w.tile([P, nblk1], F32, tag="adst1")
            nc.scalar.activation(adst1[:],
                                 dr1[:, 0:nblk1, 65:66].rearrange("p b o -> p (b o)"),
                                 AF.Identity)
            for sb_i, sb in enumerate(sbs1):
                G1 = l1w.tile([P, sb["S"], P], F32, tag="G1")
                nc.gpsimd.dma_gather(
                    G1[:], h1tab[:, :],
                    l1_eidx_s[:, 8 * sb["slot0"]:8 * (sb["slot0"] + sb["S"])],
                    sb["S"] * P, sb["S"] * P, P, single_packet=False)
                for g in [g for g in groups1 if g["sb"] == sb_i]:
                    B, L, off = g["B"], g["L"], g["slot_off"]
                    sl0 = sb["slot0"] + off
                    msg = _emit_group(
                        nc, l1w, G1[:, off:off + B * L, :],
                        l1_mask_s[:, sl0:sl0 + B * L],
                        adst1[:, g["b0"]:g["b0"] + B],
                        l1_degpos_s[:, g["b0"]:g["b0"] + B], B, L)
                    for j in range(B):
                        b = g["b0"] + j
                        mT_p = l1p.tile([D, P], F32, space="PSUM", tag="mT")
                        nc.tensor.transpose(mT_p[:], msg[:, j, :], ident[:])
                        mT_s = l1w.tile([D + 1, P], F32, tag="mTs")
                        nc.vector.tensor_copy(mT_s[0:D, :], mT_p[:])
                        nc.vector.memset(mT_s[D:D + 1, :], 1.0)
                        row_p = l1p.tile([P, D + 2], F32, space="PSUM", tag="rowp")
                        nc.tensor.matmul(row_p[:], mT_s[:], SPEC[:],
                                         start=True, stop=True)
                        row_s = l1w.tile([P, P], F32, tag="rows")
                        nc.scalar.copy(row_s[:, 0:D + 2], row_p[:])
                        nc.vector.memset(row_s[:, D + 2:P], 0.0)
                        nrows = min(P, K - b * P)
                        if nrows > 0:
                            nc.sync.dma_start(
                                tab[1 + b * P:1 + b * P + nrows, :],
                                row_s[0:nrows, :])
                        if b == K // P:   # default row from the pad position K
                            q = K % P
                            nc.sync.dma_start(tab[0:1, :], row_s[q:q + 1, :])
                            # replicate the default row over rows K+1..VTAB-1
                            # (spreads the 99%-default gather traffic across
                            # HBM addresses instead of hammering one row)
                            zidx = l1w.tile([P, 8], I16, tag="zidx")
                            nc.vector.memset(zidx[:], 0)
                            defbc = l1w.tile([P, 1, P], F32, tag="defbc")
                            nc.gpsimd.dma_gather(defbc[:], tab[:, :], zidx[:],
                                                 P, P, P, single_packet=False)
                            r0 = K + 1
                            while r0 < VTAB:
                                cnt = min(P, VTAB - r0)
                                nc.sync.dma_start(tab[r0:r0 + cnt, :],
                                                  defbc[0:cnt, 0, :])
                                r0 += cnt

        # ---- build pairfix rows: [row(a) | row(b)] for mixed pairs ----
        pfidx_s = const.tile([P, 8 * Spf], I16)
        nc.sync.dma_start(pfidx_s[:], pfidx_in[:])
        with tc.tile_pool(name="pfw", bufs=1) as pfw:
            pfg = pfw.tile([P, Spf, P], F32)
            nc.gpsimd.dma_gather(pfg[:], tab[0:VTAB, :], pfidx_s[:],
                                 Spf * P, Spf * P, P, single_packet=False)
            nc.sync.dma_start(
                tab[VTAB:VTAB + Spf * P, :].rearrange("(s p) f -> p s f", p=P),
                pfg[:])

        # ---- layer 2 ----
        tp = tab[:].rearrange("(r w) f -> r (w f)", w=PW)
        pidx_s = const.tile([P, 8 * (S2 // PW)], I16)
        nc.sync.dma_start(pidx_s[:], pidx_in[:])
        didx_s = const.tile([P, 8 * meta["dtot"]], I16)
        nc.sync.dma_start(didx_s[:], didx_in[:])
        mask_s = const.tile([P, S2], F32)
        nc.sync.dma_start(mask_s[:], mask_in[:])
        degpos_s = const.tile([P, nblk2], F32)
        nc.sync.dma_start(degpos_s[:], degpos_in[:])

        with tc.tile_pool(name="sbw", bufs=2) as sbw, \
             tc.tile_pool(name="gw", bufs=2) as gw, \
             tc.tile_pool(name="blk", bufs=3) as blk, \
             tc.tile_pool(name="psb", bufs=3, space="PSUM") as psb:
            sbs2_run = sbs2 if limit_sb is None else sbs2[:limit_sb]
            for _rep in range(repeat):
                for sb_i, sb in enumerate(sbs2_run):
                    nblk_sb = sb["b1"] - sb["b0"]
                    hS = sb["S"] // PW
                    pidx_t = pidx_s[:, 8 * (sb["slot0"] // PW):
                                    8 * (sb["slot0"] // PW + hS)]
                    G = sbw.tile([P, sb["S"], P], F32, tag="G")
                    for off in range(0, sb["S"], gchunk):
                        cs = min(gchunk, sb["S"] - off)
                        Gv = G[:, off:off + cs, :].rearrange(
                            "p (k w) f -> p k (w f)", w=PW)
                        nc.gpsimd.dma_gather(
                            Gv, tp, pidx_t[:, 8 * (off // PW):
                                           8 * ((off + cs) // PW)],
                            cs // PW * P, cs // PW * P, PW * P,
                            single_packet=False)
                    nb4 = sb["nb4"]
                    dr = sbw.tile([P, nb4, P], F32, tag="dr")
                    nc.gpsimd.dma_gather(
                        dr[:].rearrange("p (k w) f -> p k (w f)", w=PW), tp,
                        didx_s[:, 8 * sb["dcol0"]:8 * (sb["dcol0"] + nb4 // PW)],
                        nb4 // PW * P, nb4 // PW * P, PW * P,
                        single_packet=False)
                    adst = sbw.tile([P, nblk_sb], F32, tag="adst")
                    nc.scalar.activation(
                        adst[:],
                        dr[:, 0:nblk_sb, 65:66].rearrange("p b o -> p (b o)"),
                        AF.Identity)
                    if debug_lvl < 2:
                        dum = sbw.tile([P, P], F32, tag="dum")
                        nc.vector.tensor_copy(dum[:], G[:, 0, :])
                        continue
                    for g in [g for g in groups2 if g["sb"] == sb_i]:
                        B, L, off = g["B"], g["L"], g["slot_off"]
                        sl0 = sb["slot0"] + off
                        msg = _emit_group(
                            nc, gw, G[:, off:off + B * L, :],
                            mask_s[:, sl0:sl0 + B * L],
                            adst[:, g["b0"] - sb["b0"]:g["b0"] - sb["b0"] + B],
                            degpos_s[:, g["b0"]:g["b0"] + B], B, L)
                        if debug_lvl < 3:
                            dum2 = blk.tile([P, D], F32, tag="dum2")
                            nc.vector.tensor_copy(dum2[:], msg[:, 0, :])
                            continue
                        for j in range(B):
                            b = g["b0"] + j
                            mT_p = psb.tile([D, P], F32, space="PSUM", tag="mT")
                            nc.tensor.transpose(mT_p[:], msg[:, j, :], ident[:])
                            mT_s = blk.tile([D + 1, P], F32, tag="mTs")
                            nc.vector.tensor_copy(mT_s[0:D, :], mT_p[:])
                            nc.vector.memset(mT_s[D:D + 1, :], 1.0)
                            o_p = psb.tile([P, D], F32, space="PSUM", tag="op")
                            nc.tensor.matmul(o_p[:], mT_s[:], W2OUT[:],
                                             start=True, stop=True)
                            o_s = blk.tile([P, D], F32, tag="os")
                            nc.scalar.copy(o_s[:], o_p[:])
                            nc.sync.dma_start(out_t[b * P:(b + 1) * P, :], o_s[:])

    nc.compile()
    return nc


def make_in_maps(inputs, meta, l1, cores):
    x = np.ascontiguousarray(np.asarray(inputs["x"], dtype=np.float32))
    W1 = np.asarray(inputs["W1"], dtype=np.float32)
    W2 = np.asarray(inputs["W2"], dtype=np.float32)
    base = {
        "x_in": x,
        "W1_in": np.ascontiguousarray(W1),
        "W1T_in": np.ascontiguousarray(W1.T),
        "W2_in": np.ascontiguousarray(W2),
        "W2T_in": np.ascontiguousarray(W2.T),
        "av1_in": np.ascontiguousarray(np.stack(
            [np.asarray(inputs["a_src1"]), np.asarray(inputs["a_dst1"])],
            axis=1).astype(np.float32)),
        "av2_in": np.ascontiguousarray(np.stack(
            [np.asarray(inputs["a_src2"]), np.asarray(inputs["a_dst2"])],
            axis=1).astype(np.float32)),
        "b1row_in": np.asarray(inputs["b1"], dtype=np.float32).reshape(1, D),
        "b2row_in": np.asarray(inputs["b2"], dtype=np.float32).reshape(1, D),
        "b1col_in": np.asarray(inputs["b1"], dtype=np.float32).reshape(D, 1),
        "uidx_in": l1["uidx"],
        "l1_eidx_in": l1["l1_eidx"],
        "l1_didx_in": l1["l1_didx"],
        "l1_mask_in": l1["l1_mask"],
        "l1_degpos_in": l1["l1_degpos"],
    }
    in_maps = []
    for c in range(NCORES):
        m = dict(base)
        m["pidx_in"] = cores[c]["pidx"]
        m["pfidx_in"] = cores[c]["pfidx"]
        m["didx_in"] = cores[c]["didx"]
        m["mask_in"] = cores[c]["mask"]
        m["degpos_in"] = cores[c]["degpos"]
        in_maps.append(m)
    return in_maps


def unshard(results, cores):
    out = np.empty((N, D), np.float32)
    for c in range(NCORES):
        oc = results[c]["out"]
        order = cores[c]["order"]
        out[c * NPC + order] = oc[:NPC]
    return out


def kernel(**inputs):
    meta, l1, cores = prep(inputs)
    nc = build(meta, repeat=1)
    in_maps = make_in_maps(inputs, meta, l1, cores)
    res = run_bass_kernel_spmd(nc, in_maps, core_ids=list(range(NCORES)))
    return unshard(res.results, cores)



# revision 21
# speedup vs baseline: 89.6025x; 89.6025x over previous
"""GAT 2-layer encoder on 8 Trainium2 NeuronCores.

Reference computation: layer 1 = GAT conv over edge_index[:, :500] (weights W1),
layer 2 = GAT conv over edge_index[:, 500:] (weights W2).

Strategy:
  - Layer-1 output x1 differs from the default row b1 only on the <=500
    distinct dsts of the first 500 edges ("specials").  In layer 2 every edge
    whose src is non-special carries the identical feature row x1_def = b1 and
    (for a fixed dst d) the identical score c_d = leaky(sigma_def + delta_d),
    so the (deg_d - k_d) default edges of d collapse into ONE closed-form
    softmax term with weight ndef*exp(c_d): the term's feature row b1@W2 is
    folded into the output matmul as an extra stationary row scaled by a
    per-dst weight wdef.
  - Only dsts with k_d > 0 (or deg_d = 0) need device processing: ~8.5k of
    the 1.6M edges.  Every other dst's output row equals the default output
    row [b1|1] @ [W2;b2], broadcast-written from SBUF in two large DMAs that
    overlap the compute.
  - Layer 1 aggregates raw x rows (linearity: sum(alpha*(x@W1)) =
    (sum(alpha*x))@W1), so edge features are fetched with two multi-row
    indirect DMAs straight from x -- no intermediate feature table.
  - Sharding: dst-range partition across 8 cores (no collectives; layer 1 +
    table build replicated on every core, it is tiny).  Host side does index
    computation only (degree counts, grid layout, log of integer counts).
"""

import sys

sys.path.insert(0, "/opt/trn_rl_repo")

from contextlib import ExitStack

import numpy as np

import concourse.bacc as bacc
import concourse.bass as bass
import concourse.mybir as mybir
import concourse.tile as tile
from concourse.bass_utils import run_bass_kernel_spmd
from concourse.masks import make_identity

F32 = mybir.dt.float32
I16 = mybir.dt.int16
I32 = mybir.dt.int32
AF = mybir.ActivationFunctionType
OP = mybir.AluOpType

N = 100000
D = 64
NCORES = 8
NPC = N // NCORES          # dst nodes per core
P = 128
NSPLIT = 500               # first 500 edges -> layer 1
NEG_SLOPE = 0.2
EPS = 1e-16
BIG = 200.0                # score shift so padded slots underflow exp to 0.0
NPCPAD = ((NPC + P - 1) // P) * P


def _wrap16(flat):
    """int16 stream [n] (n%16==0) -> dma_gather idx tile [128, n//16]."""
    w = flat.reshape(-1, 16).T
    return np.ascontiguousarray(np.tile(w, (8, 1)).astype(np.int16))


def _groups(L):
    """Contiguous runs of equal L -> [{b0, B, L, slot_off}]."""
    slot_base = np.concatenate([[0], np.cumsum(L)])[:-1].astype(np.int64)
    out = []
    b = 0
    while b < len(L):
        b0 = b
        while b < len(L) and L[b] == L[b0]:
            b += 1
        out.append({"b0": b0, "B": b - b0, "L": L[b0],
                    "slot_off": int(slot_base[b0])})
    return out, slot_base


def prep(inputs):
    """Host-side index prep (pure index computation, no feature values)."""
    ei = np.asarray(inputs["edge_index"])
    src = ei[0].astype(np.int64)
    dst = ei[1].astype(np.int64)
    s1, d1 = src[:NSPLIT], dst[:NSPLIT]
    s2, d2 = src[NSPLIT:], dst[NSPLIT:]

    # ---- layer 1 grid over the K specials (+1 pad position -> default row) --
    specials, deg1 = np.unique(d1, return_counts=True)
    K = len(specials)
    order1 = np.argsort(-deg1, kind="stable")
    spec_by_pos = specials[order1]
    nblk1 = (K + 1 + P - 1) // P
    npos1 = nblk1 * P
    deg1s = np.zeros(npos1, np.int64)
    deg1s[:K] = deg1[order1]
    L1 = [max(int(deg1s[b * P:(b + 1) * P].max()), 1) for b in range(nblk1)]
    groups1, slot_base1 = _groups(L1)
    S1 = int(sum(L1))

    # table row of grid position q = (q%P)*nblk1 + q//P (partition-major so
    # the whole table is one DMA from a [128, nblk1, 128] SBUF tile)
    qarr = np.arange(K)
    defrow = (K % P) * nblk1 + K // P
    rowmap = np.full(N, defrow, np.int64)
    rowmap[spec_by_pos] = (qarr % P) * nblk1 + qarr // P
    TABR = nblk1 * P

    # layer-1 edge slots (values = src NODE ids for indirect x gather)
    rank1 = np.empty(K, np.int64)
    rank1[order1] = np.arange(K)
    d1pos = rank1[np.searchsorted(specials, d1)]
    pe = np.argsort(d1pos, kind="stable")
    pos_s = d1pos[pe]
    val_s = s1[pe]
    start_of_pos = np.searchsorted(pos_s, np.arange(npos1))
    kk = np.arange(len(pos_s)) - start_of_pos[pos_s]
    flat = (slot_base1[pos_s // P] + kk) * P + (pos_s % P)
    sidx1 = np.zeros(S1 * P, np.int32)
    sidx1[flat] = val_s
    l1m = np.zeros(S1 * P, np.float32)
    l1m[flat] = 1.0
    sidx1 = np.ascontiguousarray(sidx1.reshape(S1, P).T)
    l1_mask = np.ascontiguousarray(l1m.reshape(S1, P).T)
    dn = np.zeros(npos1, np.int64)
    dn[:K] = spec_by_pos
    didx1 = np.ascontiguousarray(dn.reshape(nblk1, P).T.astype(np.int32))
    dp = np.zeros(npos1, np.float32)
    dp[:K] = 1.0
    l1_degpos = np.ascontiguousarray(dp.reshape(nblk1, P).T)

    # ---- layer 2: affected dsts only (k>0 special in-edges, or deg==0) ----
    core_dat = []
    for c in range(NCORES):
        sel = (d2 >= c * NPC) & (d2 < (c + 1) * NPC)
        dl = d2[sel] - c * NPC
        sl = s2[sel]
        deg = np.bincount(dl, minlength=NPC)
        spm = rowmap[sl] != defrow
        dls = dl[spm]
        sls = sl[spm]
        kcnt = np.bincount(dls, minlength=NPC)
        aff = (kcnt > 0) | (deg == 0)
        A = np.nonzero(aff)[0]
        is_sp = rowmap[c * NPC + A] != defrow
        ordA = np.lexsort((-kcnt[A], np.logical_not(is_sp)))
        core_dat.append({"deg": deg, "k": kcnt, "A_sorted": A[ordA],
                         "nspec": int(is_sp.sum()), "dls": dls, "sls": sls,
                         "aff": aff})

    nAmax = max(len(cd["A_sorted"]) for cd in core_dat)
    nblkA = max(1, (nAmax + P - 1) // P)
    nApad = nblkA * P
    nSmax = max(cd["nspec"] for cd in core_dat)
    nSblk = min(nblkA, max(1, (nSmax + P - 1) // P))
    kprof = np.zeros(nApad, np.int64)
    for cd in core_dat:
        kk2 = cd["k"][cd["A_sorted"]]
        kprof[:len(kk2)] = np.maximum(kprof[:len(kk2)], kk2)
    L2 = [max(int(kprof[b * P:(b + 1) * P].max()), 1) for b in range(nblkA)]
    groups2, slot_base2 = _groups(L2)
    S2 = int(sum(L2))
    NROWS = nApad + NPCPAD
    Wd = NPCPAD // P

    cores = []
    for c, cd in enumerate(core_dat):
        A_sorted = cd["A_sorted"]
        nA = len(A_sorted)
        rankA = np.full(NPC, -1, np.int64)
        rankA[A_sorted] = np.arange(nA)
        pos = rankA[cd["dls"]]
        pe2 = np.argsort(pos, kind="stable")
        pos_s2 = pos[pe2]
        val2 = rowmap[cd["sls"][pe2]]
        start2 = np.searchsorted(pos_s2, np.arange(nApad))
        kk2 = np.arange(len(pos_s2)) - start2[pos_s2]
        sglob = slot_base2[pos_s2 // P] + kk2
        flat2 = sglob * P + (pos_s2 % P)
        eidxflat = np.full(S2 * P, defrow, np.int16)
        eidxflat[flat2] = val2
        maskflat = np.zeros(S2 * P, np.float32)
        maskflat[flat2] = 1.0
        mask2 = np.ascontiguousarray(maskflat.reshape(S2, P).T)
        eidx = _wrap16(eidxflat)
        # one-hot permutation [128, nSblk*nblk1*128]: for block-0.. special
        # positions p, PERM[sb][w][p_src, p] = 1 iff the dst's table row is
        # p_src*nblk1 + w  (non-special dsts select the default row)
        dvals = np.full(nSblk * P, defrow, np.int64)
        nn = min(nA, nSblk * P)
        dvals[:nn] = rowmap[c * NPC + A_sorted[:nn]]
        perm = np.zeros((nSblk, nblk1, P, P), np.float32)
        qq = np.arange(nSblk * P)
        perm[qq // P, dvals % nblk1, dvals // nblk1, qq % P] = 1.0
        perm = np.ascontiguousarray(
            perm.transpose(2, 0, 1, 3).reshape(P, nSblk * nblk1 * P))
        # per-position arrays [128, nblkA]
        degq = np.zeros(nApad, np.float32)
        pmq = np.zeros(nApad, np.float32)
        lnnq = np.zeros(nApad, np.float32)
        kq = cd["k"][A_sorted]
        ndef = cd["deg"][A_sorted] - kq
        degq[:nA] = (cd["deg"][A_sorted] > 0)
        pmq[:nA] = (ndef > 0)
        lnnq[:nA] = np.log(np.maximum(ndef, 1).astype(np.float64))
        degpos2 = np.ascontiguousarray(degq.reshape(nblkA, P).T)
        pm2 = np.ascontiguousarray(pmq.reshape(nblkA, P).T)
        lnn2 = np.ascontiguousarray(lnnq.reshape(nblkA, P).T)
        fpack = np.ascontiguousarray(np.concatenate(
            [l1_mask, l1_degpos, mask2, degpos2, pm2, lnn2, perm], axis=1))
        i16pack = eidx
        cores.append({"fpack": fpack, "i16pack": i16pack,
                      "A_sorted": A_sorted,
                      "non": np.nonzero(~cd["aff"])[0]})

    i32pack = np.ascontiguousarray(np.concatenate([sidx1, didx1], axis=1))

    meta = {
        "K": K, "nblk1": nblk1, "S1": S1, "groups1": groups1,
        "S2": S2, "nblkA": nblkA, "nSblk": nSblk, "groups2": groups2,
        "nApad": nApad, "NROWS": NROWS, "Wd": Wd, "TABR": TABR,
        "defrow": defrow,
        "FW": S1 + nblk1 + S2 + 3 * nblkA + nSblk * nblk1 * P,
        "IW32": S1 + nblk1, "IW16": 8 * S2,
    }
    l1 = {"i32pack": i32pack}
    return meta, l1, cores


def build(meta, repeat=1):
    """Build the SPMD Bass program (common across cores)."""
    nblk1, S1, groups1 = meta["nblk1"], meta["S1"], meta["groups1"]
    S2, nblkA, nSblk, groups2 = (meta["S2"], meta["nblkA"], meta["nSblk"],
                                 meta["groups2"])
    NROWS, Wd, nApad, TABR = (meta["NROWS"], meta["Wd"], meta["nApad"],
                              meta["TABR"])
    FW, IW32, IW16 = meta["FW"], meta["IW32"], meta["IW16"]
    oS1 = 0
    oDP1 = S1
    oM2 = S1 + nblk1
    oDP2 = oM2 + S2
    oPM = oDP2 + nblkA
    oLNN = oPM + nblkA
    oPRM = oLNN + nblkA

    nc = bacc.Bacc("TRN2", target_bir_lowering=False, debug=False,
                   num_devices=NCORES)
    dt = nc.dram_tensor
    x_in = dt("x_in", [N, D], F32, kind="ExternalInput").ap()
    wpack_in = dt("wpack_in", [D, 261], F32, kind="ExternalInput").ap()
    rows2_in = dt("rows2_in", [2, D], F32, kind="ExternalInput").ap()
    i32_in = dt("i32_in", [P, IW32], I32, kind="ExternalInput").ap()
    i16_in = dt("i16_in", [P, IW16], I16, kind="ExternalInput").ap()
    f_in = dt("f_in", [P, FW], F32, kind="ExternalInput").ap()
    out_t = dt("out", [NROWS, D], F32, kind="ExternalOutput").ap()
    tab = dt("tab", [TABR, P], F32).ap()
    scr = dt("scr", [1, D], F32).ap()

    with tile.TileContext(nc) as tc, ExitStack() as ctx:
        const = ctx.enter_context(tc.tile_pool(name="const", bufs=1))

        ident = const.tile([P, P], F32)
        make_identity(nc, ident[:])

        # ---- inputs (packed: 3 on SP, 2 on Act) ----
        i32s = const.tile([P, IW32], I32)
        nc.sync.dma_start(i32s[:], i32_in[:])
        wpk = const.tile([D, 261], F32)
        nc.sync.dma_start(wpk[:], wpack_in[:])
        W2OUTX = const.tile([D + 2, D + 2], F32)
        nc.vector.memset(W2OUTX[:, D:D + 2], 0.0)
        nc.sync.dma_start(W2OUTX[D:D + 1, 0:D], rows2_in[1:2, :])
        W2 = wpk[:, 128:192]
        nc.scalar.copy(W2OUTX[0:D, 0:D], W2)
        # warm the Exp table on Act before its first real use
        wtmp = const.tile([1, 1], F32)
        nc.vector.memset(wtmp[:], 0.0)
        wout = const.tile([1, 1], F32)
        nc.scalar.activation(wout[:], wtmp[:], AF.Exp)
        r2 = const.tile([2, D], F32)
        nc.sync.dma_start(r2[:], rows2_in[:])
        i16s = const.tile([P, IW16], I16)
        nc.sync.dma_start(i16s[:], i16_in[:])
        fs = const.tile([P, FW], F32)
        nc.sync.dma_start(fs[:], f_in[:])

        # ---- derived weights (setup PSUM pool, freed before main loop) ----
        W1T = wpk[:, 64:128]
        W2T = wpk[:, 192:256]
        av1 = wpk[:, 256:258]
        av2 = wpk[:, 258:260]
        b1col = wpk[:, 260:261]
        psc_ctx = tc.tile_pool(name="psc", bufs=1, space="PSUM")
        psc = psc_ctx.__enter__()
        # --- default-output-row chain first (gates the Pool broadcast fill) --
        defcol = const.tile([D + 2, 1], F32)
        nc.scalar.copy(defcol[0:D, :], b1col)
        nc.vector.memset(defcol[D:D + 2, :], 0.0)
        nc.vector.memset(defcol[D:D + 1, :], 1.0)
        defp = psc.tile([1, D + 2], F32, space="PSUM", tag="r1x")
        nc.tensor.matmul(defp[:], defcol[0:D + 1, :], W2OUTX[0:D + 1, :],
                         start=True, stop=True)
        defs_ = const.tile([1, D], F32)
        nc.scalar.copy(defs_[:], defp[:, 0:D])
        onesr = const.tile([1, P], F32)
        nc.vector.memset(onesr[:], 1.0)
        dbc_p = psc.tile([P, D], F32, space="PSUM", tag="bc")
        nc.tensor.matmul(dbc_p[:], onesr[:], defs_[:], start=True, stop=True)
        defbc = const.tile([P, 1, D], F32)
        nc.scalar.copy(defbc[:], dbc_p[:].rearrange("p (o f) -> p o f", o=1))
        # --- remaining derived weights ---
        wt1_p = psc.tile([D, 2], F32, space="PSUM", tag="v2")
        nc.tensor.matmul(wt1_p[:], W1T, av1, start=True, stop=True)
        wt1s = const.tile([D, 2], F32)
        nc.vector.tensor_copy(wt1s[:], wt1_p[:])
        wt2_p = psc.tile([D, 2], F32, space="PSUM", tag="v2")
        nc.tensor.matmul(wt2_p[:], W2T, av2, start=True, stop=True)
        wt2s = const.tile([D, 2], F32)
        nc.vector.tensor_copy(wt2s[:], wt2_p[:])
        wv1_p = psc.tile([1, D], F32, space="PSUM", tag="r1")
        nc.tensor.transpose(wv1_p[:], wt1s[:, 0:1], ident[0:D, 0:D])
        wv1 = const.tile([1, D], F32)
        nc.vector.tensor_copy(wv1[:], wv1_p[:])
        wv2_p = psc.tile([1, D], F32, space="PSUM", tag="r1")
        nc.tensor.transpose(wv2_p[:], wt1s[:, 1:2], ident[0:D, 0:D])
        wv2 = const.tile([1, D], F32)
        nc.vector.tensor_copy(wv2[:], wv2_p[:])
        wsb_p = psc.tile([P, D], F32, space="PSUM", tag="bc")
        nc.tensor.matmul(wsb_p[:], onesr[:], wv1[:], start=True, stop=True)
        w1srcb = const.tile([P, 1, D], F32)
        nc.vector.tensor_copy(w1srcb[:], wsb_p[:].rearrange("p (o f) -> p o f", o=1))
        wdb_p = psc.tile([P, D], F32, space="PSUM", tag="bc")
        nc.tensor.matmul(wdb_p[:], onesr[:], wv2[:], start=True, stop=True)
        w1dstb = const.tile([P, 1, D], F32)
        nc.vector.tensor_copy(w1dstb[:], wdb_p[:].rearrange("p (o f) -> p o f", o=1))
        b1w_p = psc.tile([1, 2], F32, space="PSUM", tag="s2")
        nc.tensor.matmul(b1w_p[:], b1col, wt2s[:], start=True, stop=True)
        b1ws = const.tile([1, 2], F32)
        nc.scalar.copy(b1ws[:], b1w_p[:])
        sd_p = psc.tile([P, 2], F32, space="PSUM", tag="p2")
        nc.tensor.matmul(sd_p[:], onesr[:], b1ws[:], start=True, stop=True)
        sdb = const.tile([P, 2], F32)
        nc.scalar.copy(sdb[:], sd_p[:])
        BIGMAT = const.tile([D + 1, D + 2], F32)
        nc.scalar.copy(BIGMAT[0:D, 0:D], wpk[:, 0:64])
        w1w2_p = psc.tile([D, 2], F32, space="PSUM", tag="v2")
        nc.tensor.matmul(w1w2_p[:], W1T, wt2s[:], start=True, stop=True)
        nc.scalar.copy(BIGMAT[0:D, D:D + 2], w1w2_p[:])
        nc.scalar.copy(BIGMAT[D:D + 1, 0:D], r2[0:1, :])
        nc.scalar.copy(BIGMAT[D:D + 1, D:D + 2], b1ws[:])
        b1W2_p = psc.tile([1, D], F32, space="PSUM", tag="r1")
        nc.tensor.matmul(b1W2_p[:], b1col, W2, start=True, stop=True)
        b1W2s = const.tile([1, D], F32)
        nc.scalar.copy(b1W2s[:], b1W2_p[:])
        nc.sync.dma_start(scr[:, :], b1W2s[:])
        nc.sync.dma_start(W2OUTX[D + 1:D + 2, 0:D], scr[:, :])
        psc_ctx.__exit__(None, None, None)

        # default-region staging tile: all three chunk DMAs read from the
        # same 48 columns (content identical); per-chunk gate columns are
        # written by data-gated ops to sequence the DMAs
        WdA = 35               # chunk A: outdef[0:35)   <- big[0:35)
        WdB = 50               # chunk B1: outdef[35:50) <- big[0:15)
        WdC = Wd - WdB         # chunk B2: outdef[50:98) <- big[0:48)
        big = const.tile([P, WdC, D], F32)
        nc.vector.tensor_copy(big[:, 0:WdA - 1, :],
                              defbc[:].to_broadcast((P, WdA - 1, D)))
        nc.vector.tensor_copy(big[:, WdA:WdC - 1, :],
                              defbc[:].to_broadcast((P, WdC - 1 - WdA, D)))

        outdef = out_t[nApad:NROWS, :].rearrange("(p w) f -> p w f", p=P)

        with tc.tile_pool(name="l1w", bufs=2) as l1w, \
             tc.tile_pool(name="gw", bufs=2) as gw, \
             tc.tile_pool(name="blk", bufs=3) as blk, \
             tc.tile_pool(name="ps", bufs=2, space="PSUM") as ps:
          for _rep in range(repeat):
            # ---- layer 1: gather dst and edge-src x rows directly ----
            # (single-column offsets: multi-column indirect is broken on HW)
            xd = l1w.tile([P, nblk1, D], F32, tag="xd")
            for b in range(nblk1):
                nc.gpsimd.indirect_dma_start(
                    out=xd[:, b, :], out_offset=None, in_=x_in[:, :],
                    in_offset=bass.IndirectOffsetOnAxis(
                        ap=i32s[:, S1 + b:S1 + b + 1], axis=0))
            L0 = groups1[0]["B"] * groups1[0]["L"]
            xg0 = l1w.tile([P, L0, D], F32, tag="xg0")
            for s in range(L0):
                nc.gpsimd.indirect_dma_start(
                    out=xg0[:, s, :], out_offset=None, in_=x_in[:, :],
                    in_offset=bass.IndirectOffsetOnAxis(
                        ap=i32s[:, s:s + 1], axis=0))
            xg1 = l1w.tile([P, max(S1 - L0, 1), D], F32, tag="xg1")
            for s in range(L0, S1):
                nc.gpsimd.indirect_dma_start(
                    out=xg1[:, s - L0, :], out_offset=None, in_=x_in[:, :],
                    in_offset=bass.IndirectOffsetOnAxis(
                        ap=i32s[:, s:s + 1], axis=0))
            # gate column WdA-1 on the dst gather tile: chunk A's DMA then
            # runs in the layer-1 compute window
            nc.vector.scalar_tensor_tensor(
                big[:, WdA - 1:WdA, :], xd[:, 0:1, :], 0.0,
                defbc[:].to_broadcast((P, 1, D)), op0=OP.mult, op1=OP.add)
            nc.sync.dma_start(outdef[:, 0:WdA, :], big[:, 0:WdA, :])
            # per-slot/per-position attention pre-activations
            a1s = l1w.tile([P, S1], F32, tag="a1s")
            t1 = l1w.tile([P, L0, D], F32, tag="t1")
            nc.vector.tensor_tensor(t1[:], xg0[:],
                                    w1srcb[:].to_broadcast((P, L0, D)),
                                    op=OP.mult)
            nc.vector.tensor_reduce(a1s[:, 0:L0], t1[:],
                                    axis=mybir.AxisListType.X, op=OP.add)
            if S1 > L0:
                t1b = l1w.tile([P, S1 - L0, D], F32, tag="t1b")
                nc.vector.tensor_tensor(
                    t1b[:], xg1[:, 0:S1 - L0, :],
                    w1srcb[:].to_broadcast((P, S1 - L0, D)), op=OP.mult)
                nc.vector.tensor_reduce(a1s[:, L0:S1], t1b[:],
                                        axis=mybir.AxisListType.X, op=OP.add)
            t2 = l1w.tile([P, nblk1, D], F32, tag="t2")
            nc.vector.tensor_tensor(t2[:], xd[:],
                                    w1dstb[:].to_broadcast((P, nblk1, D)),
                                    op=OP.mult)
            a1d = l1w.tile([P, nblk1], F32, tag="a1d")
            nc.vector.tensor_reduce(a1d[:], t2[:], axis=mybir.AxisListType.X,
                                    op=OP.add)

            tab_sb = l1w.tile([P, nblk1, P], F32, tag="tab_sb")
            nc.vector.memset(tab_sb[:, :, D + 2:P], 0.0)
            for g in groups1:
                B, L, off = g["B"], g["L"], g["slot_off"]
                BL = B * L
                s_t = gw.tile([P, B, L], F32, tag="s_t")
                nc.vector.tensor_tensor(
                    s_t[:], a1s[:, off:off + BL],
                    a1d[:, g["b0"]:g["b0"] + B].to_broadcast((P, B, L)),
                    op=OP.add)
                u_t = gw.tile([P, B, L], F32, tag="u_t")
                nc.vector.scalar_tensor_tensor(u_t[:], s_t[:], NEG_SLOPE,
                                               s_t[:], op0=OP.mult, op1=OP.max)
                e2 = gw.tile([P, B, L], F32, tag="e2")
                nc.vector.scalar_tensor_tensor(
                    e2[:], u_t[:], BIG, fs[:, oS1 + off:oS1 + off + BL],
                    op0=OP.add, op1=OP.mult)
                mx = gw.tile([P, B], F32, tag="mx")
                nc.vector.tensor_reduce(mx[:], e2[:], axis=mybir.AxisListType.X,
                                        op=OP.max)
                dd = gw.tile([P, B, L], F32, tag="dd")
                nc.vector.tensor_tensor(dd[:], e2[:],
                                        mx[:].to_broadcast((P, B, L)),
                                        op=OP.subtract)
                ex = gw.tile([P, B, L], F32, tag="ex")
                nc.scalar.activation(ex[:], dd[:], AF.Exp)
                ssum = gw.tile([P, B], F32, tag="ssum")
                nc.vector.tensor_reduce(ssum[:], ex[:],
                                        axis=mybir.AxisListType.X, op=OP.add)
                sp = gw.tile([P, B], F32, tag="sp")
                nc.vector.tensor_scalar_add(sp[:], ssum[:], EPS)
                rs = gw.tile([P, B], F32, tag="rs")
                nc.vector.reciprocal(rs[:], sp[:])
                rsd = gw.tile([P, B], F32, tag="rsd")
                nc.vector.tensor_tensor(
                    rsd[:], rs[:], fs[:, oDP1 + g["b0"]:oDP1 + g["b0"] + B],
                    op=OP.mult)
                alpha = gw.tile([P, B, L], F32, tag="alpha")
                nc.vector.tensor_tensor(alpha[:], ex[:],
                                        rsd[:].to_broadcast((P, B, L)),
                                        op=OP.mult)
                wr = gw.tile([P, BL, D], F32, tag="wr")
                xsrc = (xg0[:, off:off + BL, :] if off < L0
                        else xg1[:, off - L0:off - L0 + BL, :])
                nc.vector.tensor_tensor(
                    wr[:], xsrc,
                    alpha[:].rearrange("p b l -> p (b l)")
                    .to_broadcast((P, BL, D)), op=OP.mult)
                msgx = gw.tile([P, B, D + 1], F32, tag="msgx")
                nc.vector.memset(msgx[:, :, D:D + 1], 1.0)
                nc.vector.tensor_reduce(
                    msgx[:, :, 0:D], wr[:].rearrange("p (b l) f -> p b f l", b=B),
                    axis=mybir.AxisListType.X, op=OP.add)
                for j in range(B):
                    b = g["b0"] + j
                    tp = ps.tile([D + 2, P], F32, space="PSUM", tag="tp")
                    nc.tensor.transpose(tp[0:D + 1, :], msgx[:, j, :], ident[:])
                    mT = blk.tile([D + 1, P], F32, tag="mT1")
                    nc.vector.tensor_copy(mT[:], tp[0:D + 1, :])
                    row_p = ps.tile([P, D + 2], F32, space="PSUM", tag="acc")
                    nc.tensor.matmul(row_p[:], mT[:], BIGMAT[:],
                                     start=True, stop=True)
                    nc.scalar.copy(tab_sb[:, b, 0:D + 2], row_p[:])

            # ---- one-DMA table write (rows partition-major) ----
            nc.sync.dma_start(
                tab[:, :].rearrange("(p w) f -> p (w f)", p=P),
                tab_sb[:].rearrange("p w f -> p (w f)"))

            # ---- layer 2 gather ----
            G = l1w.tile([P, S2, P], F32, tag="G")
            nc.gpsimd.dma_gather(G[:], tab[:, :], i16s[:, 0:8 * S2],
                                 S2 * P, S2 * P, P, single_packet=False)

            # dst delta for the special-dst blocks straight from tab_sb via
            # one-hot permutation matmuls (no DRAM round-trip)
            adst = l1w.tile([P, nblkA], F32, tag="adst")
            for sb in range(nSblk):
                ad_p = ps.tile([P, 1], F32, space="PSUM", tag="ad")
                for w in range(nblk1):
                    nc.tensor.matmul(
                        ad_p[:], fs[:, oPRM + (sb * nblk1 + w) * P:
                                    oPRM + (sb * nblk1 + w + 1) * P],
                        tab_sb[:, w, 65:66],
                        start=(w == 0), stop=(w == nblk1 - 1))
                nc.scalar.copy(adst[:, sb:sb + 1], ad_p[:])
            nc.vector.scalar_tensor_tensor(
                big[:, WdB - WdA - 1:WdB - WdA, :], adst[:, 0:1].rearrange(
                    "p (b o) -> p b o", o=1).to_broadcast((P, 1, D)), 0.0,
                defbc[:].to_broadcast((P, 1, D)), op0=OP.mult, op1=OP.add)
            nc.sync.dma_start(outdef[:, WdA:WdB, :], big[:, 0:WdB - WdA, :])

            # gate column Wd-1 on the G gather: the last chunk's transfer
            # then overlaps the layer-2 compute tail instead of delaying G
            nc.vector.scalar_tensor_tensor(
                big[:, WdC - 1:WdC, :], G[:, 0:1, 0:D], 0.0,
                defbc[:].to_broadcast((P, 1, D)), op0=OP.mult, op1=OP.add)
            nc.sync.dma_start(outdef[:, WdB:Wd, :], big[:, 0:WdC, :])

            # ---- per-position dst terms ----
            if nblkA > nSblk:
                nc.vector.tensor_copy(
                    adst[:, nSblk:nblkA],
                    sdb[:, 1:2].to_broadcast((P, nblkA - nSblk)))
            smt = l1w.tile([P, nblkA], F32, tag="smt")
            nc.vector.tensor_tensor(smt[:], adst[:],
                                    sdb[:, 0:1].to_broadcast((P, nblkA)),
                                    op=OP.add)
            ck = l1w.tile([P, nblkA], F32, tag="ck")
            nc.vector.scalar_tensor_tensor(ck[:], smt[:], NEG_SLOPE, smt[:],
                                           op0=OP.mult, op1=OP.max)
            ck2 = l1w.tile([P, nblkA], F32, tag="ck2")
            nc.vector.tensor_tensor(ck2[:], ck[:], fs[:, oLNN:oLNN + nblkA],
                                    op=OP.add)
            cb = l1w.tile([P, nblkA], F32, tag="cb")
            nc.vector.scalar_tensor_tensor(cb[:], ck2[:], BIG,
                                           fs[:, oPM:oPM + nblkA],
                                           op0=OP.add, op1=OP.mult)

            o_all = l1w.tile([P, nblkA, D], F32, tag="o_all")
            for g in groups2:
                B, L, off = g["B"], g["L"], g["slot_off"]
                BL = B * L
                b0 = g["b0"]
                asrc = G[:, off:off + BL, 64:65].rearrange("p s o -> p (s o)")
                s_t = gw.tile([P, B, L], F32, tag="s_t2")
                nc.vector.tensor_tensor(
                    s_t[:], asrc, adst[:, b0:b0 + B].to_broadcast((P, B, L)),
                    op=OP.add)
                u_t = gw.tile([P, B, L], F32, tag="u_t2")
                nc.vector.scalar_tensor_tensor(u_t[:], s_t[:], NEG_SLOPE,
                                               s_t[:], op0=OP.mult, op1=OP.max)
                e2 = gw.tile([P, B, L], F32, tag="e2b")
                nc.vector.scalar_tensor_tensor(
                    e2[:], u_t[:], BIG, fs[:, oM2 + off:oM2 + off + BL],
                    op0=OP.add, op1=OP.mult)
                mx = gw.tile([P, B], F32, tag="mxb")
                nc.vector.tensor_reduce(mx[:], e2[:], axis=mybir.AxisListType.X,
                                        op=OP.max)
                mm2 = gw.tile([P, B], F32, tag="mm2")
                nc.vector.tensor_tensor(mm2[:], mx[:], cb[:, b0:b0 + B],
                                        op=OP.max)
                dd = gw.tile([P, B, L], F32, tag="ddb")
                nc.vector.tensor_tensor(dd[:], e2[:],
                                        mm2[:].to_broadcast((P, B, L)),
                                        op=OP.subtract)
                ex = gw.tile([P, B, L], F32, tag="exb")
                nc.scalar.activation(ex[:], dd[:], AF.Exp)
                zd0 = gw.tile([P, B], F32, tag="zd0")
                nc.vector.tensor_tensor(zd0[:], cb[:, b0:b0 + B], mm2[:],
                                        op=OP.subtract)
                zd1 = gw.tile([P, B], F32, tag="zd1")
                nc.scalar.activation(zd1[:], zd0[:], AF.Exp)
                zdef = gw.tile([P, B], F32, tag="zdef")
                nc.vector.tensor_tensor(zdef[:], zd1[:],
                                        fs[:, oPM + b0:oPM + b0 + B],
                                        op=OP.mult)
                ssum = gw.tile([P, B], F32, tag="ssumb")
                nc.vector.tensor_reduce(ssum[:], ex[:],
                                        axis=mybir.AxisListType.X, op=OP.add)
                Z = gw.tile([P, B], F32, tag="Z")
                nc.vector.tensor_tensor(Z[:], ssum[:], zdef[:], op=OP.add)
                sp = gw.tile([P, B], F32, tag="spb")
                nc.vector.tensor_scalar_add(sp[:], Z[:], EPS)
                rs = gw.tile([P, B], F32, tag="rsb")
                nc.vector.reciprocal(rs[:], sp[:])
                rsd = gw.tile([P, B], F32, tag="rsdb")
                nc.vector.tensor_tensor(
                    rsd[:], rs[:], fs[:, oDP2 + b0:oDP2 + b0 + B], op=OP.mult)
                alpha = gw.tile([P, B, L], F32, tag="alphab")
                nc.vector.tensor_tensor(alpha[:], ex[:],
                                        rsd[:].to_broadcast((P, B, L)),
                                        op=OP.mult)
                msg = gw.tile([P, B, D + 2], F32, tag="msgb")
                nc.vector.memset(msg[:, :, D:D + 1], 1.0)
                wdef = gw.tile([P, B], F32, tag="wdef")
                nc.vector.tensor_tensor(wdef[:], zdef[:], rsd[:], op=OP.mult)
                nc.vector.tensor_copy(msg[:, :, D + 1:D + 2],
                                      wdef[:].rearrange("p (b o) -> p b o", o=1))
                wr = gw.tile([P, BL, D], F32, tag="wrb")
                nc.vector.tensor_tensor(
                    wr[:], G[:, off:off + BL, 0:D],
                    alpha[:].rearrange("p b l -> p (b l)")
                    .to_broadcast((P, BL, D)), op=OP.mult)
                nc.vector.tensor_reduce(
                    msg[:, :, 0:D], wr[:].rearrange("p (b l) f -> p b f l", b=B),
                    axis=mybir.AxisListType.X, op=OP.add)
                for j in range(B):
                    b = b0 + j
                    tp = ps.tile([D + 2, P], F32, space="PSUM", tag="tp")
                    nc.tensor.transpose(tp[:], msg[:, j, :], ident[:])
                    mT = blk.tile([D + 2, P], F32, tag="mT2")
                    nc.vector.tensor_copy(mT[:], tp[:])
                    o_p = ps.tile([P, D + 2], F32, space="PSUM", tag="acc")
                    nc.tensor.matmul(o_p[:], mT[:], W2OUTX[:],
                                     start=True, stop=True)
                    nc.scalar.copy(o_all[:, b, :], o_p[:, 0:D])

            # ---- one-DMA affected-region write (rows partition-major) ----
            nc.scalar.dma_start(
                out_t[0:nApad, :].rearrange("(p w) f -> p (w f)", p=P),
                o_all[:].rearrange("p w f -> p (w f)"))

    nc.compile()
    return nc


def make_in_maps(inputs, meta, l1, cores):
    x = np.ascontiguousarray(np.asarray(inputs["x"], dtype=np.float32))
    W1 = np.asarray(inputs["W1"], dtype=np.float32)
    W2 = np.asarray(inputs["W2"], dtype=np.float32)
    wpack = np.concatenate([
        W1, W1.T, W2, W2.T,
        np.stack([np.asarray(inputs["a_src1"]), np.asarray(inputs["a_dst1"])],
                 axis=1).astype(np.float32),
        np.stack([np.asarray(inputs["a_src2"]), np.asarray(inputs["a_dst2"])],
                 axis=1).astype(np.float32),
        np.asarray(inputs["b1"], dtype=np.float32).reshape(D, 1),
    ], axis=1)
    rows2 = np.stack([np.asarray(inputs["b1"], dtype=np.float32),
                      np.asarray(inputs["b2"], dtype=np.float32)])
    base = {
        "x_in": x,
        "wpack_in": np.ascontiguousarray(wpack),
        "rows2_in": np.ascontiguousarray(rows2),
        "i32_in": l1["i32pack"],
    }
    in_maps = []
    for c in range(NCORES):
        m = dict(base)
        m["i16_in"] = cores[c]["i16pack"]
        m["f_in"] = cores[c]["fpack"]
        in_maps.append(m)
    return in_maps


def unshard_core(oc, core, meta):
    nApad, nblkA = meta["nApad"], meta["nblkA"]
    A_sorted = core["A_sorted"]
    non = core["non"]
    got = np.empty((NPC, D), np.float32)
    q = np.arange(len(A_sorted))
    got[A_sorted] = oc[(q % P) * nblkA + q // P]
    got[non] = oc[nApad:nApad + len(non)]
    return got


def unshard(results, cores, meta):
    out = np.empty((N, D), np.float32)
    for c in range(NCORES):
        out[c * NPC:(c + 1) * NPC] = unshard_core(
            np.asarray(results[c]["out"]), cores[c], meta)
    return out


def kernel(**inputs):
    meta, l1, cores = prep(inputs)
    nc = build(meta, repeat=1)
    in_maps = make_in_maps(inputs, meta, l1, cores)
    res = run_bass_kernel_spmd(nc, in_maps, core_ids=list(range(NCORES)))
    return unshard(res.results, cores, meta)
